# revision 7
# baseline (speedup 1.0000x reference)
"""Trainium2 Bass kernel for nn_CustomGate: apply a DxD single-qudit gate M
along tensor axis `index` of a (N, B) state batch.

Math: x viewed as (left, D, right, B); out[a,i,r,b] = sum_j M[i,j] * x[a,j,r,b].
For the spec'd problem: N=2^24, B=2, D=2, index=5 -> left=32, right=2^18.

Sharding: split the leading `left` axis across 8 cores (contiguous row chunks
of x). The gate contraction is then fully local per core; M is replicated.

Per-core layout (f32 flat): [A pairs, D=2, 64, F] where a slab (a, j) is a
contiguous 64*F-element block. Two `a`-slabs are stacked to form full
128-partition tiles:
    U = [s0_a ; s0_a'] (j=0), V = [s1_a ; s1_a'] (j=1)
    Y0 = m00*U + m01*V   (output j=0 slabs)
    Y1 = m10*U + m11*V   (output j=1 slabs)
computed as ACT mul (scale from SBUF) + DVE scalar_tensor_tensor in-place.
"""

import os

import numpy as np

N_CORES = 8
P = 128  # SBUF partitions

_BUILD_CACHE = {}

# knobs (overridable via env for tuning)
FS = int(os.environ.get("GATE_FS", "4096"))  # free-dim chunk per tile
BUFS = int(os.environ.get("GATE_BUFS", "2"))  # tile-pool buffers

LAST_RESULT = None  # test.py reads profiling info from here


def _build_nc(pairs_per_core: int, slab_elems: int):
    """Build the Bass/Tile program for one core.

    pairs_per_core: number of `a` values per core (must be even).
    slab_elems: elements in one (a, j) slab = right * B. Must divide by 64.
    """
    import concourse.bacc as bacc
    import concourse.mybir as mybir
    import concourse.tile as tile

    F = slab_elems // P  # free dim when one slab fills all 128 partitions
    fs = min(FS, F)
    assert F % fs == 0
    n_fchunks = F // fs

    nc = bacc.Bacc(trn_type="TRN2", target_bir_lowering=False)
    xs = nc.dram_tensor(
        "xs", [pairs_per_core, 2, P, F], mybir.dt.float32, kind="ExternalInput"
    ).ap()
    m = nc.dram_tensor("m", [2, 2], mybir.dt.float32, kind="ExternalInput").ap()
    ys = nc.dram_tensor(
        "ys", [pairs_per_core, 2, P, F], mybir.dt.float32, kind="ExternalOutput"
    ).ap()

    with tile.TileContext(nc) as tc:
        with (
            tc.tile_pool(name="const", bufs=1) as cpool,
            tc.tile_pool(name="io", bufs=BUFS) as pool,
        ):
            # broadcast M's 4 scalars across all 128 partitions: mb[p, k]
            mb = cpool.tile([P, 4], mybir.dt.float32)
            nc.sync.dma_start(
                out=mb[:, :],
                in_=m.rearrange("a b -> (a b)").unsqueeze(0).to_broadcast((P, 4)),
            )

            for a in range(pairs_per_core):
                for c in range(n_fchunks):
                    cs = c * fs
                    u = pool.tile([P, fs], mybir.dt.float32)
                    v = pool.tile([P, fs], mybir.dt.float32)
                    nc.sync.dma_start(out=u[:, :], in_=xs[a, 0, :, cs : cs + fs])
                    nc.sync.dma_start(out=v[:, :], in_=xs[a, 1, :, cs : cs + fs])

                    y0 = pool.tile([P, fs], mybir.dt.float32)
                    y1 = pool.tile([P, fs], mybir.dt.float32)
                    # ACT: y = m00*U / m10*U
                    nc.scalar.mul(y0[:, :], u[:, :], mb[:, 0:1])
                    nc.scalar.mul(y1[:, :], u[:, :], mb[:, 2:3])
                    # DVE: y += m01*V / m11*V  (in-place on in1)
                    nc.vector.scalar_tensor_tensor(
                        out=y0[:, :],
                        in0=v[:, :],
                        scalar=mb[:, 1:2],
                        in1=y0[:, :],
                        op0=mybir.AluOpType.mult,
                        op1=mybir.AluOpType.add,
                    )
                    nc.vector.scalar_tensor_tensor(
                        out=y1[:, :],
                        in0=v[:, :],
                        scalar=mb[:, 3:4],
                        in1=y1[:, :],
                        op0=mybir.AluOpType.mult,
                        op1=mybir.AluOpType.add,
                    )

                    nc.sync.dma_start(out=ys[a, 0, :, cs : cs + fs], in_=y0[:, :])
                    nc.sync.dma_start(out=ys[a, 1, :, cs : cs + fs], in_=y1[:, :])
    nc.compile()
    return nc


def _numpy_fallback(x, M, index, D):
    N, B = x.shape
    L = round(np.log(N) / np.log(D))
    left = D**index
    right = N // (left * D)
    xr = x.reshape(left, D, right, B)
    out = np.einsum("ij,ajrb->airb", M, xr)
    return out.reshape(N, B).astype(x.dtype)


def kernel(x, M, index, D, **_unused):
    global LAST_RESULT
    x = np.ascontiguousarray(np.asarray(x), dtype=np.float32)
    M = np.ascontiguousarray(np.asarray(M), dtype=np.float32)
    index = int(index)
    D = int(D)
    N, B = x.shape
    left = D**index
    right = N // (left * D)
    slab_elems = right * B

    ok = (
        D == 2
        and left % N_CORES == 0
        and slab_elems % 128 == 0
        and (slab_elems // 128) % 512 == 0
    )
    if not ok:
        return _numpy_fallback(x, M, index, D)

    pairs_per_core = left // N_CORES
    key = (pairs_per_core, slab_elems)
    if key not in _BUILD_CACHE:
        _BUILD_CACHE[key] = _build_nc(pairs_per_core, slab_elems)
    nc = _BUILD_CACHE[key]

    from concourse.bass_utils import run_bass_kernel_spmd

    F = slab_elems // 128
    chunk_rows = N // N_CORES
    xr = x.reshape(N_CORES, pairs_per_core, 2, 128, F)
    in_maps = [{"xs": xr[i], "m": M} for i in range(N_CORES)]
    trace = bool(os.environ.get("GATE_TRACE"))
    res = run_bass_kernel_spmd(
        nc,
        in_maps,
        core_ids=list(range(N_CORES)),
        trace=trace,
        trace_cores=[0] if trace else None,
    )
    LAST_RESULT = res
    out = np.empty((N, B), dtype=np.float32)
    ov = out.reshape(N_CORES, chunk_rows, B)
    for i in range(N_CORES):
        ov[i] = res.results[i]["ys"].reshape(chunk_rows, B)
    return out


# revision 10
# speedup vs baseline: 5.2212x; 5.2212x over previous
"""Trainium2 Bass kernel for nn_CustomGate: apply a DxD single-qudit gate M
along tensor axis `index` of a (N, B) state batch.

Math: x viewed as (left, D, right, B); out[a,i,r,b] = sum_j M[i,j] * x[a,j,r,b].
For the spec'd problem: N=2^24, B=2, D=2, index=5 -> left=32, right=2^18.

Sharding: split the leading `left` axis across 8 cores (contiguous row chunks
of x). The gate contraction is then fully local per core; M is replicated.

Per-core layout (f32 flat): [A pairs, D=2, 64, F] where a slab (a, j) is a
contiguous 64*F-element block. Two `a`-slabs are stacked to form full
128-partition tiles:
    U = [s0_a ; s0_a'] (j=0), V = [s1_a ; s1_a'] (j=1)
    Y0 = m00*U + m01*V   (output j=0 slabs)
    Y1 = m10*U + m11*V   (output j=1 slabs)
computed as ACT mul (scale from SBUF) + DVE scalar_tensor_tensor in-place.
"""

import os

import numpy as np

N_CORES = 8
P = 128  # SBUF partitions

_BUILD_CACHE = {}

# knobs (overridable via env for tuning)
FS = int(os.environ.get("GATE_FS", "4096"))  # free-dim chunk per tile
BUFS = int(os.environ.get("GATE_BUFS", "2"))  # tile-pool buffers

LAST_RESULT = None  # test.py reads profiling info from here


def _build_nc(pairs_per_core: int, slab_elems: int, repeat: int = 1):
    """Build the Bass/Tile program for one core.

    pairs_per_core: number of `a` values per core (must be even).
    slab_elems: elements in one (a, j) slab = right * B. Must divide by 64.
    """
    import concourse.bacc as bacc
    import concourse.mybir as mybir
    import concourse.tile as tile

    F = slab_elems // P  # free dim when one slab fills all 128 partitions
    fs = min(FS, F)
    assert F % fs == 0
    n_fchunks = F // fs

    nc = bacc.Bacc(trn_type="TRN2", target_bir_lowering=False)
    xs = nc.dram_tensor(
        "xs", [pairs_per_core, 2, P, F], mybir.dt.float32, kind="ExternalInput"
    ).ap()
    m = nc.dram_tensor("m", [2, 2], mybir.dt.float32, kind="ExternalInput").ap()
    ys = nc.dram_tensor(
        "ys", [pairs_per_core, 2, P, F], mybir.dt.float32, kind="ExternalOutput"
    ).ap()

    with tile.TileContext(nc) as tc:
        with (
            tc.tile_pool(name="const", bufs=1) as cpool,
            tc.tile_pool(name="io", bufs=BUFS) as pool,
        ):
            # broadcast M's 4 scalars across all 128 partitions: mb[p, k]
            mb = cpool.tile([P, 4], mybir.dt.float32)
            nc.sync.dma_start(
                out=mb[:, :],
                in_=m.rearrange("a b -> (a b)").unsqueeze(0).to_broadcast((P, 4)),
            )

            for _rep in range(repeat):
                for a in range(pairs_per_core):
                    for c in range(n_fchunks):
                        cs = c * fs
                        u = pool.tile([P, fs], mybir.dt.float32)
                        v = pool.tile([P, fs], mybir.dt.float32)
                        nc.sync.dma_start(out=u[:, :], in_=xs[a, 0, :, cs : cs + fs])
                        nc.sync.dma_start(out=v[:, :], in_=xs[a, 1, :, cs : cs + fs])

                        y0 = pool.tile([P, fs], mybir.dt.float32)
                        y1 = pool.tile([P, fs], mybir.dt.float32)
                        # ACT: y = m00*U / m10*U
                        nc.scalar.mul(y0[:, :], u[:, :], mb[:, 0:1])
                        nc.scalar.mul(y1[:, :], u[:, :], mb[:, 2:3])
                        # DVE: y += m01*V / m11*V  (in-place on in1)
                        nc.vector.scalar_tensor_tensor(
                            out=y0[:, :],
                            in0=v[:, :],
                            scalar=mb[:, 1:2],
                            in1=y0[:, :],
                            op0=mybir.AluOpType.mult,
                            op1=mybir.AluOpType.add,
                        )
                        nc.vector.scalar_tensor_tensor(
                            out=y1[:, :],
                            in0=v[:, :],
                            scalar=mb[:, 3:4],
                            in1=y1[:, :],
                            op0=mybir.AluOpType.mult,
                            op1=mybir.AluOpType.add,
                        )

                        nc.sync.dma_start(out=ys[a, 0, :, cs : cs + fs], in_=y0[:, :])
                        nc.sync.dma_start(out=ys[a, 1, :, cs : cs + fs], in_=y1[:, :])
    nc.compile()
    return nc


def _numpy_fallback(x, M, index, D):
    N, B = x.shape
    L = round(np.log(N) / np.log(D))
    left = D**index
    right = N // (left * D)
    xr = x.reshape(left, D, right, B)
    out = np.einsum("ij,ajrb->airb", M, xr)
    return out.reshape(N, B).astype(x.dtype)


def kernel(x, M, index, D, **_unused):
    global LAST_RESULT
    x = np.ascontiguousarray(np.asarray(x), dtype=np.float32)
    M = np.ascontiguousarray(np.asarray(M), dtype=np.float32)
    index = int(index)
    D = int(D)
    N, B = x.shape
    left = D**index
    right = N // (left * D)
    slab_elems = right * B

    ok = (
        D == 2
        and left % N_CORES == 0
        and slab_elems % 128 == 0
        and (slab_elems // 128) % 512 == 0
    )
    if not ok:
        return _numpy_fallback(x, M, index, D)

    pairs_per_core = left // N_CORES
    key = (pairs_per_core, slab_elems)
    if key not in _BUILD_CACHE:
        _BUILD_CACHE[key] = _build_nc(pairs_per_core, slab_elems)
    nc = _BUILD_CACHE[key]

    from concourse.bass_utils import run_bass_kernel_spmd

    F = slab_elems // 128
    chunk_rows = N // N_CORES
    xr = x.reshape(N_CORES, pairs_per_core, 2, 128, F)
    in_maps = [{"xs": xr[i], "m": M} for i in range(N_CORES)]
    trace = bool(os.environ.get("GATE_TRACE"))
    res = run_bass_kernel_spmd(
        nc,
        in_maps,
        core_ids=list(range(N_CORES)),
        trace=trace,
        trace_cores=[0] if trace else None,
    )
    LAST_RESULT = res
    out = np.empty((N, B), dtype=np.float32)
    ov = out.reshape(N_CORES, chunk_rows, B)
    for i in range(N_CORES):
        ov[i] = res.results[i]["ys"].reshape(chunk_rows, B)
    return out
